# revision 1
# baseline (speedup 1.0000x reference)
"""JKNet (3x GraphConv+LN+ReLU, JK-concat, Linear, LN) on 8 Trainium2 cores.

Strategy: edges partitioned by dst across 8 cores; h table replicated per
core (fp16) and AllGather'd between layers. Per-edge scalars
q_e = ew * rsqrt(deg_out[src]) * rsqrt(deg_in[dst]) folded on host into
fp16 selection tiles S[e, v_local]; aggregation = per-block accumulating
matmuls  Z^T[96,128v] += G_t[128e,96]^T @ S_t[128e,128v]  with G_t rows
gathered from the h table by indirect DMA. Per-core nodes are LPT
bin-packed into 49 blocks of 128 to equalize block degree (output rows
inverse-permuted on host at the end).
"""

import math
import heapq

import numpy as np

N = 50000
E = 800000
D = 96
DOUT = 64
NCORES = 8
CHUNK = N // NCORES          # 6250
P = 128
NB = math.ceil(CHUNK / P)    # 49 blocks (last has 106 rows)
EPS = 1e-5
F16 = np.float16


def _host_preprocess(x, src, dst, edge_weight):
    src = np.asarray(src).astype(np.int64)
    dst = np.asarray(dst).astype(np.int64)
    ew = np.asarray(edge_weight).astype(np.float32)
    x = np.asarray(x).astype(np.float32)

    deg_out = np.maximum(np.bincount(src, minlength=N), 1).astype(np.float32)
    deg_in = np.maximum(np.bincount(dst, minlength=N), 1).astype(np.float32)
    q = ew / (np.sqrt(deg_out[src]) * np.sqrt(deg_in[dst]))

    # Assign node v -> (core, block, row). LPT bin-packing of each core's
    # nodes into NB blocks (capacity 128, last 106) by in-degree.
    pos = np.empty(N, dtype=np.int64)
    core_of = np.empty(N, dtype=np.int32)
    blk_of = np.empty(N, dtype=np.int32)
    row_of = np.empty(N, dtype=np.int32)
    for c in range(NCORES):
        nodes = np.arange(c * CHUNK, (c + 1) * CHUNK)
        order = np.argsort(-deg_in[nodes], kind="stable")
        caps = [P] * (NB - 1) + [CHUNK - P * (NB - 1)]
        heap = [(0.0, b) for b in range(NB)]
        heapq.heapify(heap)
        fill = [0] * NB
        for v in nodes[order]:
            while True:
                load, b = heapq.heappop(heap)
                if fill[b] < caps[b]:
                    break
            r = fill[b]
            fill[b] += 1
            if fill[b] < caps[b]:
                heapq.heappush(heap, (load + deg_in[v], b))
            core_of[v] = c
            blk_of[v] = b
            row_of[v] = r
            pos[v] = c * CHUNK + b * P + r

    # Per-core edge slots.
    e_core = core_of[dst]
    e_blk = blk_of[dst]
    e_row = row_of[dst]
    src_pos = pos[src]

    per_core = []
    kts = []
    for c in range(NCORES):
        sel = np.nonzero(e_core == c)[0]
        b_e = e_blk[sel]
        order = np.argsort(b_e, kind="stable")
        sel = sel[order]
        b_e = b_e[order]
        counts = np.bincount(b_e, minlength=NB)
        kts.append(int(np.ceil(counts.max() / P)))
        offs = np.zeros(NB + 1, dtype=np.int64)
        np.cumsum(counts, out=offs[1:])
        rank = np.arange(len(sel)) - offs[b_e]
        per_core.append((sel, b_e, rank))
    KT = max(kts + [1])

    ins = []
    for c in range(NCORES):
        sel, b_e, rank = per_core[c]
        dest = b_e.astype(np.int64) * (KT * P) + rank
        srcflat = np.zeros(NB * KT * P, dtype=np.int32)
        srcflat[dest] = src_pos[sel].astype(np.int32)
        sflat = np.zeros(NB * KT * P * P, dtype=F16)
        sflat[dest * P + e_row[sel]] = q[sel].astype(F16)
        ins.append(
            {
                "sidx": np.ascontiguousarray(
                    srcflat.reshape(NB, KT, P).transpose(0, 2, 1)
                ),
                "sm": np.ascontiguousarray(
                    sflat.reshape(NB, KT, P, P)
                    .transpose(0, 2, 1, 3)
                    .reshape(NB, P, KT * P)
                ),
            }
        )

    xp = np.zeros((N, D), dtype=F16)
    xp[pos] = x.astype(F16)
    return ins, xp, pos, KT


def _build_bass(KT):
    import concourse.bacc as bacc
    import concourse.bass as bass
    import concourse.mybir as mybir
    import concourse.tile as tile
    from concourse.masks import make_identity

    dt = mybir.dt
    Alu = mybir.AluOpType
    Act = mybir.ActivationFunctionType

    nc = bacc.Bacc(
        "TRN2", target_bir_lowering=False, debug=False, num_devices=NCORES
    )

    h0 = nc.dram_tensor("h0", [N, D], dt.float16, kind="ExternalInput")
    sidx = nc.dram_tensor("sidx", [NB, P, KT], dt.int32, kind="ExternalInput")
    sm = nc.dram_tensor("sm", [NB, P, KT * P], dt.float16, kind="ExternalInput")
    ws = [
        nc.dram_tensor(f"w{l}", [D, D], dt.float32, kind="ExternalInput")
        for l in range(3)
    ]
    wo = nc.dram_tensor("wo", [3 * D, DOUT], dt.float32, kind="ExternalInput")
    cbs = [
        nc.dram_tensor(f"cb{l}", [1, D], dt.float32, kind="ExternalInput")
        for l in range(3)
    ]
    gs = [
        nc.dram_tensor(f"g{l}", [1, D], dt.float32, kind="ExternalInput")
        for l in range(3)
    ]
    bes = [
        nc.dram_tensor(f"be{l}", [1, D], dt.float32, kind="ExternalInput")
        for l in range(3)
    ]
    bo = nc.dram_tensor("bo", [1, DOUT], dt.float32, kind="ExternalInput")
    go = nc.dram_tensor("go", [1, DOUT], dt.float32, kind="ExternalInput")
    beo = nc.dram_tensor("beo", [1, DOUT], dt.float32, kind="ExternalInput")
    out = nc.dram_tensor("out", [CHUNK, DOUT], dt.float32, kind="ExternalOutput")

    from contextlib import ExitStack

    with tile.TileContext(nc) as tc, ExitStack() as ctx:
        cpool = ctx.enter_context(tc.tile_pool(name="const", bufs=1))
        wpool = ctx.enter_context(tc.tile_pool(name="work", bufs=3))
        ppool2 = ctx.enter_context(tc.tile_pool(name="ps2", bufs=2, space="PSUM"))
        ppool1 = ctx.enter_context(tc.tile_pool(name="ps1", bufs=1, space="PSUM"))
        dram = ctx.enter_context(tc.tile_pool(name="dram", bufs=1, space="DRAM"))

        h_loc = [
            dram.tile([CHUNK, D], dt.float16, name=f"hloc{l}") for l in range(2)
        ]
        h_full = [
            dram.tile([N, D], dt.float16, addr_space="Shared", name=f"hfull{l}")
            for l in range(2)
        ]

        id128f = cpool.tile([P, P], dt.float32, name="id128f")
        make_identity(nc, id128f[:])
        id128h = cpool.tile([P, P], dt.float16, name="id128h")
        make_identity(nc, id128h[:])
        id96f = cpool.tile([D, D], dt.float32, name="id96f")
        make_identity(nc, id96f[:])
        id64f = cpool.tile([DOUT, DOUT], dt.float32, name="id64f")
        make_identity(nc, id64f[:])
        zero1 = cpool.tile([P, 1], dt.float32, name="zero1")
        nc.vector.memset(zero1[:], 0.0)
        eps1 = cpool.tile([P, 1], dt.float32, name="eps1")
        nc.vector.memset(eps1[:], EPS)

        def bcast(src_ap, width, name):
            row = cpool.tile([1, width], dt.float32, name=name + "_r")
            nc.sync.dma_start(out=row[:], in_=src_ap)
            t = cpool.tile([P, width], dt.float32, name=name)
            nc.gpsimd.partition_broadcast(t[:], row[:])
            return t

        w_sb = []
        for l in range(3):
            t = cpool.tile([D, D], dt.float32, name=f"wsb{l}")
            nc.sync.dma_start(out=t[:], in_=ws[l][:])
            w_sb.append(t)
        wo_sb = []
        for k in range(3):
            t = cpool.tile([D, DOUT], dt.float32, name=f"wosb{k}")
            nc.sync.dma_start(out=t[:], in_=wo[k * D : (k + 1) * D, :])
            wo_sb.append(t)
        cb_bc = [bcast(cbs[l][:], D, f"cbbc{l}") for l in range(3)]
        g_bc = [bcast(gs[l][:], D, f"gbc{l}") for l in range(3)]
        be_bc = [bcast(bes[l][:], D, f"bebc{l}") for l in range(3)]
        bo_bc = bcast(bo[:], DOUT, "bobc")
        go_bc = bcast(go[:], DOUT, "gobc")
        beo_bc = bcast(beo[:], DOUT, "beobc")

        def layer_norm(cn_ps, width, bias_t, gt, bt, relu, out_dtype):
            """cn_ps: PSUM [P, width] pre-bias. Returns SBUF [P, width]."""
            cn = wpool.tile([P, width], dt.float32, tag="lncn")
            nc.vector.tensor_tensor(
                out=cn[:], in0=cn_ps[:], in1=bias_t[:], op=Alu.add
            )
            mu_s = wpool.tile([P, 1], dt.float32, tag="lnmus")
            nc.vector.tensor_reduce(
                out=mu_s[:], in_=cn[:], axis=mybir.AxisListType.X, op=Alu.add
            )
            mu = wpool.tile([P, 1], dt.float32, tag="lnmu")
            nc.scalar.activation(
                out=mu[:], in_=mu_s[:], func=Act.Copy, scale=1.0 / width
            )
            d = wpool.tile([P, width], dt.float32, tag="lnd")
            nc.vector.tensor_scalar_sub(out=d[:], in0=cn[:], scalar1=mu[:, :1])
            sq = wpool.tile([P, width], dt.float32, tag="lnsq")
            ssq = wpool.tile([P, 1], dt.float32, tag="lnssq")
            nc.scalar.activation(
                out=sq[:], in_=d[:], func=Act.Square, accum_out=ssq[:],
                bias=zero1[:, :1],
            )
            std = wpool.tile([P, 1], dt.float32, tag="lnstd")
            nc.scalar.activation(
                out=std[:], in_=ssq[:], func=Act.Sqrt, scale=1.0 / width,
                bias=eps1[:, :1],
            )
            rstd = wpool.tile([P, 1], dt.float32, tag="lnrstd")
            nc.vector.reciprocal(out=rstd[:], in_=std[:])
            hn = wpool.tile([P, width], dt.float32, tag="lnhn")
            nc.vector.tensor_scalar_mul(out=hn[:], in0=d[:], scalar1=rstd[:, :1])
            hg = wpool.tile([P, width], dt.float32, tag="lnhg")
            nc.vector.tensor_tensor(out=hg[:], in0=hn[:], in1=gt[:], op=Alu.mult)
            hf = wpool.tile([P, width], out_dtype, tag="lnhf")
            if relu:
                hb = wpool.tile([P, width], dt.float32, tag="lnhb")
                nc.vector.tensor_tensor(
                    out=hb[:], in0=hg[:], in1=bt[:], op=Alu.add
                )
                nc.vector.tensor_scalar_max(out=hf[:], in0=hb[:], scalar1=0.0)
            else:
                nc.vector.tensor_tensor(
                    out=hf[:], in0=hg[:], in1=bt[:], op=Alu.add
                )
            return hf

        for l in range(3):
            table = h0 if l == 0 else h_full[l - 1]
            relu = l < 2
            for b in range(NB):
                vr = P if b < NB - 1 else CHUNK - P * (NB - 1)
                rows = slice(b * P, b * P + vr)

                idx = wpool.tile([P, KT], dt.int32, tag="idx")
                nc.sync.dma_start(out=idx[:], in_=sidx[b])
                s_blk = wpool.tile([P, KT * P], dt.float16, tag="sblk")
                nc.sync.dma_start(out=s_blk[:], in_=sm[b])
                z_ps = ppool2.tile([D, P], dt.float32, tag="z", space="PSUM")
                for t in range(KT):
                    gt = wpool.tile([P, D], dt.float16, tag="gt", bufs=6)
                    nc.gpsimd.indirect_dma_start(
                        out=gt[:],
                        out_offset=None,
                        in_=table[:],
                        in_offset=bass.IndirectOffsetOnAxis(
                            ap=idx[:, t : t + 1], axis=0
                        ),
                    )
                    nc.tensor.matmul(
                        out=z_ps[:],
                        lhsT=gt[:],
                        rhs=s_blk[:, t * P : (t + 1) * P],
                        start=(t == 0),
                        stop=(t == KT - 1),
                    )
                zt = wpool.tile([D, P], dt.float32, tag="zt")
                nc.vector.tensor_copy(out=zt[:], in_=z_ps[:])
                c_ps = ppool1.tile([D, P], dt.float32, tag="c", space="PSUM")
                nc.tensor.matmul(
                    out=c_ps[:], lhsT=w_sb[l][:], rhs=zt[:], start=True, stop=True
                )
                c_sb = wpool.tile([D, P], dt.float32, tag="csb")
                nc.vector.tensor_copy(out=c_sb[:], in_=c_ps[:])
                cn_ps = ppool1.tile([P, D], dt.float32, tag="cn", space="PSUM")
                nc.tensor.transpose(out=cn_ps[:], in_=c_sb[:], identity=id96f[:])

                hf = layer_norm(
                    cn_ps,
                    D,
                    cb_bc[l],
                    g_bc[l],
                    be_bc[l],
                    relu,
                    dt.float32,
                )

                if l < 2:
                    h16 = wpool.tile([P, D], dt.float16, tag="h16")
                    nc.vector.tensor_copy(out=h16[:], in_=hf[:])
                    nc.sync.dma_start(out=h_loc[l][rows], in_=h16[:vr])
                else:
                    # fused JK: concat(h1,h2,h3) @ Wo + bo -> LN -> out
                    f_ps = ppool1.tile([DOUT, P], dt.float32, tag="f", space="PSUM")
                    for k in range(3):
                        tk_ps = ppool1.tile([D, P], dt.float32, tag="t", space="PSUM")
                        if k < 2:
                            hk = wpool.tile([P, D], dt.float16, tag="hk")
                            if vr < P:
                                nc.vector.memset(hk[:], 0)
                            nc.sync.dma_start(out=hk[:vr], in_=h_loc[k][rows])
                            hk32 = wpool.tile([P, D], dt.float32, tag="hk32")
                            nc.vector.tensor_copy(out=hk32[:], in_=hk[:])
                            nc.tensor.transpose(
                                out=tk_ps[:], in_=hk32[:], identity=id128f[:]
                            )
                        else:
                            nc.tensor.transpose(
                                out=tk_ps[:], in_=hf[:], identity=id128f[:]
                            )
                        tk = wpool.tile([D, P], dt.float32, tag="tk")
                        nc.vector.tensor_copy(out=tk[:], in_=tk_ps[:])
                        nc.tensor.matmul(
                            out=f_ps[:],
                            lhsT=wo_sb[k][:],
                            rhs=tk[:],
                            start=(k == 0),
                            stop=(k == 2),
                        )
                    f_sb = wpool.tile([DOUT, P], dt.float32, tag="fsb")
                    nc.vector.tensor_copy(out=f_sb[:], in_=f_ps[:])
                    fn_ps = ppool1.tile([P, DOUT], dt.float32, tag="fn", space="PSUM")
                    nc.tensor.transpose(out=fn_ps[:], in_=f_sb[:], identity=id64f[:])
                    fo = layer_norm(
                        fn_ps, DOUT, bo_bc, go_bc, beo_bc, False, dt.float32
                    )
                    nc.sync.dma_start(out=out[rows], in_=fo[:vr])

            if l < 2:
                nc.gpsimd.collective_compute(
                    "AllGather",
                    Alu.bypass,
                    ins=[h_loc[l][:]],
                    outs=[h_full[l][:]],
                    replica_groups=[list(range(NCORES))],
                )

    nc.finalize()
    return nc


_CACHE = {}


def kernel(
    x, src, dst, edge_weight,
    W0, b0, g0, be0, W1, b1, g1, be1, W2, b2, g2, be2,
    Wo, bo, go, beo,
):
    from concourse import bass_utils

    per_core, xp, pos, KT = _host_preprocess(x, src, dst, edge_weight)

    if KT not in _CACHE:
        _CACHE[KT] = _build_bass(KT)
    nc = _CACHE[KT]

    common = {
        "h0": xp,
        "w0": np.asarray(W0, np.float32),
        "w1": np.asarray(W1, np.float32),
        "w2": np.asarray(W2, np.float32),
        "wo": np.asarray(Wo, np.float32),
        "cb0": np.asarray(b0, np.float32).reshape(1, D),
        "cb1": np.asarray(b1, np.float32).reshape(1, D),
        "cb2": np.asarray(b2, np.float32).reshape(1, D),
        "g0": np.asarray(g0, np.float32).reshape(1, D),
        "g1": np.asarray(g1, np.float32).reshape(1, D),
        "g2": np.asarray(g2, np.float32).reshape(1, D),
        "be0": np.asarray(be0, np.float32).reshape(1, D),
        "be1": np.asarray(be1, np.float32).reshape(1, D),
        "be2": np.asarray(be2, np.float32).reshape(1, D),
        "bo": np.asarray(bo, np.float32).reshape(1, DOUT),
        "go": np.asarray(go, np.float32).reshape(1, DOUT),
        "beo": np.asarray(beo, np.float32).reshape(1, DOUT),
    }
    in_maps = [dict(common, **per_core[c]) for c in range(NCORES)]

    import os

    res = bass_utils.run_bass_kernel_spmd(
        nc,
        in_maps,
        core_ids=list(range(NCORES)),
        trace=bool(os.environ.get("BASS_TRACE")),
    )
    y_perm = np.concatenate([r["out"] for r in res.results], axis=0)
    if res.exec_time_ns is not None:
        kernel.last_exec_time_ns = res.exec_time_ns
    kernel.last_results = res
    return y_perm[pos].astype(np.float32)

